# revision 7
# baseline (speedup 1.0000x reference)
"""Trainium2 Bass kernel for causal multi-head attention.

Reference computation (B=2, T=2048, D=1024, H=16 heads, head_dim=64):
    q, k, v = x @ Wq, x @ Wk, x @ Wv         (per-head split)
    out = softmax(causal(q k^T / 8)) v  @ Wo

Sharding: 8 cores = 2 batches x 4 head-groups (4 heads each).  Each core
computes, for its batch b and its 4 heads:
    qT, kT [256, 2048] and v [2048, 256]  from the host-pre-transposed xT,
    transposed scores sT[tk, tq] = kT.T @ qT  (so softmax sums land on the
    matmul contraction axis and no on-chip transposes are ever needed),
    expS = exp(sT/8) * causal_mask,
    ctxT' [65, tq] = v'.T @ expS   with v' = [v | ones] so row 64 is the
    softmax denominator,
    ctxT_norm = ctxT * (1/rowsum)  (rank-1 PE broadcast of the reciprocal),
    partial_out [2048, 1024] = ctxT.T @ Wo[g*256:(g+1)*256, :].
Host sums the 4 partials per batch.

All matmuls run as float32r (TF32-like, full PE rate at N>=256).  Tiles that
feed the PE are allocated as float32r (walrus requires producer dtype to
match); PSUM accumulation stays fp32.
"""

import sys

if "/opt/trn_rl_repo" not in sys.path:
    sys.path.insert(0, "/opt/trn_rl_repo")

import numpy as np

B, T, D, H = 2, 2048, 1024, 16
HD = 64                   # head dim
NCORES = 8
GROUPS = 4                # head groups (cores per batch)
HPC = H // GROUPS         # heads per core = 4
DHC = HPC * HD            # per-core head columns = 256
NKB = D // 128            # 8 contraction blocks for the projections
NTB = T // 128            # 16 t-blocks
NCH = T // 512            # 4 tq chunks of 512

_CACHE = {}


def _build():
    import concourse.bacc as bacc
    import concourse.tile as tile
    from concourse import mybir

    fp32 = mybir.dt.float32
    fp32r = mybir.dt.float32r
    Exp = mybir.ActivationFunctionType.Exp

    nc = bacc.Bacc("TRN2", target_bir_lowering=False, debug=False,
                   num_devices=NCORES)

    xt_d = nc.dram_tensor("xt", [D, T], fp32, kind="ExternalInput")
    wq_d = nc.dram_tensor("wq", [D, DHC], fp32, kind="ExternalInput")
    wk_d = nc.dram_tensor("wk", [D, DHC], fp32, kind="ExternalInput")
    wv_d = nc.dram_tensor("wv", [D, DHC], fp32, kind="ExternalInput")
    wo_d = nc.dram_tensor("wo", [DHC, D], fp32, kind="ExternalInput")
    cm_d = nc.dram_tensor("cmask", [128, 1024], fp32, kind="ExternalInput")
    out_d = nc.dram_tensor("out", [T, D], fp32, kind="ExternalOutput")

    with tile.TileContext(nc) as tc:
        with (
            tc.tile_pool(name="consts", bufs=1) as consts,
            tc.tile_pool(name="big", bufs=1) as big,
            tc.tile_pool(name="es_pool", bufs=4) as es_pool,
            tc.tile_pool(name="small", bufs=4) as small,
            tc.tile_pool(name="outp", bufs=2) as outp,
            tc.tile_pool(name="psum", bufs=1, space="PSUM") as psum,
        ):
            wq_sb = consts.tile([128, NKB, DHC], fp32r)
            nc.sync.dma_start(out=wq_sb, in_=wq_d[:].rearrange("(k p) n -> p k n", p=128).bitcast(fp32r))
            wk_sb = consts.tile([128, NKB, DHC], fp32r)
            nc.sync.dma_start(out=wk_sb, in_=wk_d[:].rearrange("(k p) n -> p k n", p=128).bitcast(fp32r))
            wv_sb = consts.tile([128, NKB, DHC], fp32r)
            nc.sync.dma_start(out=wv_sb, in_=wv_d[:].rearrange("(k p) n -> p k n", p=128).bitcast(fp32r))
            wo_sb = consts.tile([128, 2, D], fp32r)
            nc.sync.dma_start(out=wo_sb, in_=wo_d[:].rearrange("(k p) n -> p k n", p=128).bitcast(fp32r))
            cm_sb = consts.tile([128, 1024], fp32r)
            nc.sync.dma_start(out=cm_sb, in_=cm_d[:].bitcast(fp32r))

            xt_sb = big.tile([128, NKB, T], fp32r)
            qt_sb = big.tile([128, 2, T], fp32r)
            kt_sb = big.tile([128, 2, T], fp32r)
            ct_sb = big.tile([128, 2, T], fp32r)
            vs_sb = big.tile([128, NTB, HPC, HD + 1], fp32r)
            # ones column of v' (cmask cols 512.. are all 1.0, dtype fp32r)
            nc.vector.tensor_copy(
                vs_sb[:, :, :, 64],
                cm_sb[:, 512:512 + NTB * HPC].rearrange("p (a b) -> p a b", a=NTB),
            )

            xt_r = xt_d[:].rearrange("(k p) t -> p k t", p=128).bitcast(fp32r)

            for nj in range(NCH):
                cs = slice(nj * 512, (nj + 1) * 512)
                nc.sync.dma_start(out=xt_sb[:, :, cs], in_=xt_r[:, :, cs])

                # ---- QKV projections for this t-chunk ----
                for wsb, dst in ((wq_sb, qt_sb), (wk_sb, kt_sb)):
                    for mb in range(2):
                        pq = psum.tile([128, 512], fp32, tag="mm", bufs=4,
                                       name=f"pq{nj}{mb}")
                        for kb in range(NKB):
                            nc.tensor.matmul(
                                pq,
                                wsb[:, kb, mb * 128:(mb + 1) * 128],
                                xt_sb[:, kb, cs],
                                start=(kb == 0), stop=(kb == NKB - 1),
                            )
                        nc.vector.tensor_copy(dst[:, mb, cs], pq)
                for tb in range(4 * nj, 4 * nj + 4):
                    pv = psum.tile([128, 512], fp32, tag="mm", bufs=4,
                                   name=f"pv{tb}")
                    for kb in range(NKB):
                        nc.tensor.matmul(
                            pv[:, 0:DHC],
                            xt_sb[:, kb, tb * 128:(tb + 1) * 128],
                            wv_sb[:, kb, :],
                            start=(kb == 0), stop=(kb == NKB - 1),
                        )
                    nc.vector.tensor_copy(
                        vs_sb[:, tb, :, 0:HD],
                        pv[:, 0:DHC].rearrange("p (h d) -> p h d", h=HPC),
                    )

                # ---- attention for tq-chunk nj, all 4 heads ----
                nb = 4 * nj + 4     # causal: tk-blocks 0 .. nb-1
                for h in range(HPC):
                    mbh, ro = h >> 1, (h & 1) * 64
                    pc = psum.tile([65, 512], fp32, tag="acc", bufs=2,
                                   name=f"pc{nj}{h}")
                    for i in range(nb):
                        m = i - 4 * nj
                        # causal window: diagonal blocks only need cols >= wm
                        # (m==3 keeps N>=256 to stay at full fp32r rate)
                        wm = 0 if m < 0 else (128 * m if m < 3 else 256)
                        ps = psum.tile([128, 512], fp32, tag="mm", bufs=4,
                                       name=f"ps{nj}{h}{i}")
                        nc.tensor.matmul(
                            ps[:, wm:512],
                            kt_sb[ro:ro + 64, mbh, i * 128:(i + 1) * 128],
                            qt_sb[ro:ro + 64, mbh, nj * 512 + wm:(nj + 1) * 512],
                            start=True, stop=True,
                        )
                        es = es_pool.tile([128, 512], fp32r, tag="es",
                                          name=f"es{nj}{h}{i}")
                        nc.scalar.activation(out=es[:, wm:512], in_=ps[:, wm:512],
                                             func=Exp, scale=0.125)
                        if m >= 0:
                            nc.vector.tensor_mul(
                                es[:, wm:512], es[:, wm:512],
                                cm_sb[:, (3 - m) * 128 + wm:(3 - m) * 128 + 512],
                            )
                        nc.tensor.matmul(
                            pc[:, wm:512],
                            vs_sb[:, i, h, :],
                            es[:, wm:512],
                            start=(i == 0), stop=(i == nb - 1),
                        )
                    rc = small.tile([1, 512], fp32r, tag="rc", name=f"rc{nj}{h}")
                    with nc.allow_low_precision(reason="fp32r recip feeds fp32r PE bcast"):
                        nc.vector.reciprocal(rc, pc[64:65, :])
                    pb = psum.tile([64, 512], fp32, tag="bc", bufs=2,
                                   name=f"pb{nj}{h}")
                    nc.tensor.matmul(pb, cm_sb[0:1, 512:576], rc,
                                     start=True, stop=True)
                    bc_sb = small.tile([64, 512], fp32, tag="bc_sb",
                                       name=f"bc{nj}{h}")
                    nc.vector.tensor_copy(bc_sb, pb)
                    nc.vector.tensor_mul(ct_sb[ro:ro + 64, mbh, cs],
                                         pc[0:64, :], bc_sb)

                # ---- output projection for this chunk's t-blocks ----
                for tb in range(4 * nj, 4 * nj + 4):
                    ot = outp.tile([128, D], fp32, tag="ot", name=f"ot{tb}")
                    for nk in range(2):
                        po = psum.tile([128, 512], fp32, tag="mm", bufs=4,
                                       name=f"po{tb}{nk}")
                        for mb in range(2):
                            nc.tensor.matmul(
                                po,
                                ct_sb[:, mb, tb * 128:(tb + 1) * 128],
                                wo_sb[:, mb, nk * 512:(nk + 1) * 512],
                                start=(mb == 0), stop=(mb == 1),
                            )
                        nc.vector.tensor_copy(ot[:, nk * 512:(nk + 1) * 512], po)
                    nc.sync.dma_start(out=out_d[tb * 128:(tb + 1) * 128, :], in_=ot)

    nc.compile()
    return nc


def _causal_mask_block():
    # [128, 1024]: cols 0..383 = 0, cols 384..511 = upper-tri (p <= c-384),
    # cols 512.. = 1.  Slice [(3-m)*128 : (3-m)*128+512] masks a diagonal
    # tk-block at position m within a 512-wide tq chunk.
    m = np.zeros((128, 1024), np.float32)
    m[:, 512:] = 1.0
    m[:, 384:512] = np.triu(np.ones((128, 128), np.float32))
    return m


def _prepare_in_maps(x_q, Wq, Wk, Wv, Wo):
    x_q = np.asarray(x_q, np.float32)
    Wq = np.asarray(Wq, np.float32)
    Wk = np.asarray(Wk, np.float32)
    Wv = np.asarray(Wv, np.float32)
    Wo = np.asarray(Wo, np.float32)

    cmask = _causal_mask_block()
    xts = [np.ascontiguousarray(x_q[b].T) for b in range(B)]
    in_maps = []
    for c in range(NCORES):
        b, g = divmod(c, GROUPS)
        sl = slice(g * DHC, (g + 1) * DHC)
        in_maps.append({
            "xt": xts[b],
            "wq": np.ascontiguousarray(Wq[:, sl]),
            "wk": np.ascontiguousarray(Wk[:, sl]),
            "wv": np.ascontiguousarray(Wv[:, sl]),
            "wo": np.ascontiguousarray(Wo[sl, :]),
            "cmask": cmask,
        })
    return in_maps


def _gather(results):
    out = np.zeros((B, T, D), np.float32)
    for c in range(NCORES):
        out[c // GROUPS] += results[c]["out"]
    return out


def get_nc():
    if "nc" not in _CACHE:
        _CACHE["nc"] = _build()
    return _CACHE["nc"]


def kernel(x_q, Wq, Wk, Wv, Wo):
    from concourse.bass_utils import run_bass_kernel_spmd

    nc = get_nc()
    in_maps = _prepare_in_maps(x_q, Wq, Wk, Wv, Wo)
    res = run_bass_kernel_spmd(nc, in_maps, list(range(NCORES)))
    return _gather(res.results)
